# revision 1
# baseline (speedup 1.0000x reference)
"""Multi-head attention (B=8, N=1024, D=512, H=8) on 8 TRN2 NeuronCores.

Sharding: pure batch-parallel — core i computes batch i end-to-end, no
collectives. Host-side prep per batch: gather valid keys (mask) into a
contiguous buffer padded to NKV=640, pre-transpose x, convert streams to
bf16, and build small per-partition bias/validity tables. Device pipeline
(bf16 matmuls, f32 PSUM accumulation):
  k^T/q^T/v projections -> per head pair, scores s^T[k,q] land in one
  [128,1024] PSUM tile (head A cols 0:512 on PE row tile 0, head B cols
  512:1024 on row tile 64, overlapping on disjoint PE rows) -> one exp on
  ACT per query-half with the key-padding mask folded into the activation
  bias -> attn@v with an augmented ones-column producing the softmax
  denominator in row 64 -> fast reciprocal + Pool partition-broadcast ->
  normalize -> out-projection.

Math shortcuts: bk is dropped (constant-in-key terms cancel in softmax);
bv is folded into the output bias on the host (bob' = bo + bv @ wo since
normalized attention rows sum to 1).
"""

import sys

import numpy as np

sys.path.insert(0, "/opt/trn_rl_repo")

B, N, D, H = 8, 1024, 512, 8
HD = D // H            # 64
SCALE = HD ** -0.5     # 0.125
NKV = 640              # padded valid-key count (5 chunks of 128)
KC = NKV // 128        # 5
DC = D // 128          # 4
VW = HD + 2            # 66: aug head stride, 4B-aligned for bf16 weights
PAD_BIAS = -30000.0    # exp(PAD_BIAS + s*SCALE) == 0.0 exactly

_prog_cache = {}


def _build_program():
    import concourse.bacc as bacc
    import concourse.tile as tile
    from concourse import mybir

    dt = mybir.dt
    f32 = dt.float32
    bf16 = dt.bfloat16
    AF = mybir.ActivationFunctionType

    nc = bacc.Bacc("TRN2", target_bir_lowering=False, debug=False)

    xT_d = nc.dram_tensor("xT", [D, N], bf16, kind="ExternalInput").ap()
    xkT_d = nc.dram_tensor("xkT", [D, NKV], bf16, kind="ExternalInput").ap()
    wq_d = nc.dram_tensor("wq", [D, D], bf16, kind="ExternalInput").ap()
    wk_d = nc.dram_tensor("wk", [D, D], bf16, kind="ExternalInput").ap()
    wv_d = nc.dram_tensor("wv", [D, D], bf16, kind="ExternalInput").ap()
    wo_d = nc.dram_tensor("wo", [D, D], bf16, kind="ExternalInput").ap()
    bq_d = nc.dram_tensor("bq", [D, 1], f32, kind="ExternalInput").ap()
    bob_d = nc.dram_tensor("bob", [128, D], f32, kind="ExternalInput").ap()
    expb_d = nc.dram_tensor("expb", [128, KC], f32, kind="ExternalInput").ap()
    onesv_d = nc.dram_tensor("onesv", [128, H, 1], bf16,
                             kind="ExternalInput").ap()
    y_d = nc.dram_tensor("y", [N, D], f32, kind="ExternalOutput").ap()

    with tile.TileContext(nc) as tc, \
         nc.allow_low_precision(reason="bf16 matmul streams, f32 accumulate"):
        with tc.tile_pool(name="const", bufs=1) as cpool:
            # Persistent result tiles (vaug gets its ones column via DMA)
            qT_t = [cpool.tile([128, N], bf16, name=f"qT_t{c}")
                    for c in range(DC)]
            kT_t = [cpool.tile([128, NKV], bf16, name=f"kT_t{c}")
                    for c in range(DC)]
            vaug_t = [cpool.tile([128, H, VW], bf16, name=f"vaug_t{c}")
                      for c in range(KC)]
            aoT_t = [cpool.tile([128, N], bf16, name=f"aoT_t{c}")
                     for c in range(DC)]

            # DMA issue order = priority order (queues drain round-robin).
            # k-projection inputs first (PE starts on them), tiny tables,
            # then q inputs, v, and wo/bob last. Big loads split into
            # partition halves so they spread across two queues.
            def load_half(t, dram_slice):
                nc.sync.dma_start(t[0:64, :], dram_slice[0:64, :])
                nc.sync.dma_start(t[64:128, :], dram_slice[64:128, :])

            def alloc_w(name):
                return [cpool.tile([128, D], bf16, name=f"{name}{c}")
                        for c in range(DC)]

            wk_t, wq_t, wv_t, wo_t = (alloc_w(n) for n in
                                      ("wk_t", "wq_t", "wv_t", "wo_t"))
            xkT_t = [cpool.tile([128, NKV], bf16, name=f"xkT_t{c}")
                     for c in range(DC)]
            xT_t = [cpool.tile([128, N], bf16, name=f"xT_t{c}")
                    for c in range(DC)]

            for c in range(DC):
                load_half(wk_t[c], wk_d[128 * c:128 * (c + 1), :])
                load_half(xkT_t[c], xkT_d[128 * c:128 * (c + 1), :])
            bq_t = cpool.tile([128, DC], f32, name="bq_t")
            for c in range(DC):
                nc.sync.dma_start(bq_t[:, c:c + 1],
                                  bq_d[128 * c:128 * (c + 1), :])
            expb_t = cpool.tile([128, KC], f32, name="expb_t")
            nc.sync.dma_start(expb_t[:], expb_d[:, :])
            for c in range(KC):
                nc.sync.dma_start(vaug_t[c][:, :, HD:HD + 1], onesv_d[:])
            for c in range(DC):
                load_half(wq_t[c], wq_d[128 * c:128 * (c + 1), :])
                load_half(xT_t[c], xT_d[128 * c:128 * (c + 1), :])
            for c in range(DC):
                load_half(wv_t[c], wv_d[128 * c:128 * (c + 1), :])
            for c in range(DC):
                load_half(wo_t[c], wo_d[128 * c:128 * (c + 1), :])
            bob_t = cpool.tile([128, D], f32, name="bob_t")
            nc.sync.dma_start(bob_t[:], bob_d[:, :])

            # ---- Phase 1a: k projection (no bias: cancels in softmax) ----
            with tc.tile_pool(name="kpp", bufs=2, space="PSUM") as kpp:
                for dp in range(DC):
                    ps = kpp.tile([128, NKV], f32, name="kps")
                    for dc in range(DC):
                        lhs = wk_t[dc][:, 128 * dp:128 * (dp + 1)]
                        nc.tensor.matmul(
                            ps[:, 0:512], lhs, xkT_t[dc][:, 0:512],
                            start=(dc == 0), stop=(dc == DC - 1),
                        )
                        nc.tensor.matmul(
                            ps[:, 512:NKV], lhs, xkT_t[dc][:, 512:NKV],
                            start=(dc == 0), stop=(dc == DC - 1),
                        )
                    nc.vector.tensor_scalar_add(kT_t[dp][:], ps[:], 0.0)

            # ---- Phase 1b: q projection ----
            with tc.tile_pool(name="qpp", bufs=2, space="PSUM") as qpp:
                for dp in range(DC):
                    ps = qpp.tile([128, N], f32, name="qps")
                    for dc in range(DC):
                        lhs = wq_t[dc][:, 128 * dp:128 * (dp + 1)]
                        for hf in range(2):
                            nc.tensor.matmul(
                                ps[:, 512 * hf:512 * (hf + 1)],
                                lhs,
                                xT_t[dc][:, 512 * hf:512 * (hf + 1)],
                                start=(dc == 0), stop=(dc == DC - 1),
                            )
                    nc.vector.tensor_scalar_add(qT_t[dp][:], ps[:],
                                                bq_t[:, dp:dp + 1])

            # ---- Phase 1c: v projection (no bias: folded into bob') ----
            with tc.tile_pool(name="vpp", bufs=2, space="PSUM") as vpp:
                for c in range(KC):
                    ps = vpp.tile([128, H, HD], f32, name="vps")
                    for dc in range(DC):
                        nc.tensor.matmul(
                            ps[:], xkT_t[dc][:, 128 * c:128 * (c + 1)],
                            wv_t[dc][:],
                            start=(dc == 0), stop=(dc == DC - 1),
                        )
                    nc.vector.tensor_scalar_add(vaug_t[c][:, :, 0:HD],
                                                ps[:], 0.0)

            # ---- Phase 2: attention on head pairs. Heads A=2dp (kT/qT
            # rows 0:64) and B=2dp+1 (rows 64:128) write one [128,1024]
            # score tile per query-half; the shared exp makes both matmuls
            # feed one consumer so they stay adjacent in the PE stream and
            # overlap on disjoint PE row tiles (0,0)/(64,0).
            with tc.tile_pool(name="scp", bufs=2, space="PSUM") as scp, \
                 tc.tile_pool(name="oap", bufs=4, space="PSUM") as oap, \
                 tc.tile_pool(name="pp", bufs=6) as pp, \
                 tc.tile_pool(name="rcp", bufs=6) as rcp:
                for dp in range(DC):
                    heads = (2 * dp, 2 * dp + 1)
                    oa = {h: [oap.tile([HD + 1, 512], f32, name="oa")
                              for hf in range(2)] for h in heads}
                    p_t = []  # per chunk: [p_hf0, p_hf1]

                    def av(cav):
                        for hf in range(2):
                            for hi, h in enumerate(heads):
                                nc.tensor.matmul(
                                    oa[h][hf][:],
                                    vaug_t[cav][:, h, 0:HD + 1],
                                    p_t[cav][hf][:, 512 * hi:512 * (hi + 1)],
                                    start=(cav == 0), stop=(cav == KC - 1),
                                )

                    for c in range(KC):
                        ps = []
                        for hf in range(2):
                            sc = scp.tile([128, N], f32, name="sc")
                            for hi, h in enumerate(heads):
                                row = HD * (h % 2)
                                nc.tensor.matmul(
                                    sc[:, 512 * hi:512 * (hi + 1)],
                                    kT_t[dp][row:row + HD,
                                             128 * c:128 * (c + 1)],
                                    qT_t[dp][row:row + HD,
                                             512 * hf:512 * (hf + 1)],
                                    start=True, stop=True,
                                )
                            p = pp.tile([128, N], bf16, name="p")
                            nc.scalar.activation(
                                p[:], sc[:], AF.Exp,
                                bias=expb_t[:, c:c + 1], scale=SCALE,
                            )
                            ps.append(p)
                        p_t.append(ps)
                        if c >= 1:
                            av(c - 1)
                    av(KC - 1)

                    for h in heads:
                        for hf in range(2):
                            row = HD * (h % 2)
                            # custom DVE ops read garbage from PSUM on HW:
                            # stage the denominator row through SBUF first
                            db = rcp.tile([1, 512], f32, name="db")
                            nc.vector.tensor_scalar_add(
                                db[:], oa[h][hf][HD:HD + 1, :], 0.0)
                            rc = rcp.tile([1, 512], f32, name="rc")
                            nc.vector.reciprocal_approx_fast(rc[:], db[:])
                            rbs = rcp.tile([HD, 512], f32, name="rbs")
                            nc.gpsimd.partition_broadcast(rbs[:], rc[:])
                            nc.vector.tensor_mul(
                                aoT_t[dp][row:row + HD,
                                          512 * hf:512 * (hf + 1)],
                                oa[h][hf][0:HD, :], rbs[:])

            # ---- Phase 3: output projection ----
            with tc.tile_pool(name="ypp", bufs=2, space="PSUM") as ypp, \
                 tc.tile_pool(name="ysp", bufs=2) as ysp:
                for ic in range(N // 128):
                    yps = ypp.tile([128, D], f32, name="yps")
                    for dp in range(DC):
                        nc.tensor.matmul(
                            yps[:], aoT_t[dp][:, 128 * ic:128 * (ic + 1)],
                            wo_t[dp][:],
                            start=(dp == 0), stop=(dp == DC - 1),
                        )
                    ysb = ysp.tile([128, D], f32, name="ysb")
                    nc.vector.tensor_add(ysb[:], yps[:], bob_t[:])
                    nc.sync.dma_start(y_d[128 * ic:128 * (ic + 1), :], ysb[:])

    return nc


def _get_program():
    if "nc" not in _prog_cache:
        nc = _build_program()
        if not nc.is_finalized():
            nc.finalize()
        _prog_cache["nc"] = nc
    return _prog_cache["nc"]


def _prep_core(b, x, mask, wq, bq, wk, bk, wv, bv, wo, bo):
    import ml_dtypes

    b16 = ml_dtypes.bfloat16
    xb = np.ascontiguousarray(x[b], dtype=np.float32)       # [N, D]
    idx = np.nonzero(mask[b])[0]
    nv = int(idx.size)
    assert 1 <= nv <= NKV, f"batch {b}: {nv} valid keys, NKV={NKV}"
    xk = np.zeros((NKV, D), np.float32)
    xk[:nv] = xb[idx]
    pos = np.arange(128)[:, None] + 128 * np.arange(KC)[None, :]
    expb = np.where(pos < nv, 0.0, PAD_BIAS).astype(np.float32)
    f = np.float32
    bob = (bo.astype(f) + bv.astype(f) @ wo.astype(f)).reshape(D)
    return {
        "xT": np.ascontiguousarray(xb.T).astype(b16),
        "xkT": np.ascontiguousarray(xk.T).astype(b16),
        "wq": np.ascontiguousarray(wq, f).astype(b16),
        "wk": np.ascontiguousarray(wk, f).astype(b16),
        "wv": np.ascontiguousarray(wv, f).astype(b16),
        "wo": np.ascontiguousarray(wo, f).astype(b16),
        "bq": np.ascontiguousarray(bq, f).reshape(D, 1),
        "bob": np.ascontiguousarray(np.broadcast_to(bob, (128, D))),
        "expb": expb,
        "onesv": np.ones((128, H, 1), b16),
    }


def _run(inputs):
    import os

    os.environ["BASS_NEVER_TRACE"] = "1"
    from concourse.bass_utils import run_bass_kernel_spmd

    nc = _get_program()
    in_maps = [_prep_core(b, **inputs) for b in range(B)]
    res = run_bass_kernel_spmd(nc, in_maps, core_ids=list(range(B)),
                               trace=False)
    out = np.stack([res.results[b]["y"] for b in range(B)], axis=0)
    return out.astype(np.float32), res


def kernel(**inputs) -> np.ndarray:
    out, _ = _run(inputs)
    return out



# revision 21
# speedup vs baseline: 1.2187x; 1.2187x over previous
"""Multi-head attention (B=8, N=1024, D=512, H=8) on 8 TRN2 NeuronCores.

Sharding: pure batch-parallel - core i computes batch i end-to-end, no
collectives. Host-side prep per batch: gather valid keys (mask) into a
contiguous buffer padded to NKV=640, pre-transpose x, convert streams to
bf16, and pack all device inputs into 5 grouped DRAM tensors so the whole
input set loads with 5 large dma_starts issued in priority order on one
queue (sequential completion: kproj inputs land first).

Device pipeline (bf16 matmuls, f32 PSUM):
  PE warmup (dummy matmuls on zeros during the DMA flight, so the HAM
  clock gate opens before real work) -> k projection -> q projection for
  head-pair 0 -> attention over head pairs. Scores for heads (2dp,2dp+1)
  land in one [128,1024] PSUM tile per (chunk, query-half) via row-packed
  matmul pairs on disjoint PE row groups; one exp per tile with the
  key-padding mask folded into the activation bias; attn@v as col-packed
  M=64 matmul pairs into one [128,512] PSUM tile; softmax denominators
  via four concurrent M=1 ones-matmuls into one shared PSUM bank.
  Remaining projections (v, q for pairs 1-3) are emitted as fillers
  inside the attention loop to use PE slack while the ACT engine (the
  bottleneck: 40 exps) streams. Normalize = reciprocal + gpsimd
  partition-broadcast + one DVE multiply per (pair, half). Tail: output
  projection with bf16 DMA writeback (host upcasts).

Math shortcuts: bk dropped (cancels in softmax); bv folded into the
output bias on the host (bob' = bo + bv @ wo).
"""

import sys

import numpy as np

sys.path.insert(0, "/opt/trn_rl_repo")

B, N, D, H = 8, 1024, 512, 8
HD = D // H            # 64
SCALE = HD ** -0.5     # 0.125
NKV = 640              # padded valid-key count (5 chunks of 128)
KC = NKV // 128        # 5
DC = D // 128          # 4
PAD_BIAS = -30000.0    # exp(PAD_BIAS + s*SCALE) == 0.0 exactly

# g4 column layout (f32): bq columns, expb columns, bob block
G4_BQ = 0
G4_EXPB = DC
G4_BOB = DC + KC
G4_W = DC + KC + D

_DEBUG = False  # extra dbg output tensor with intermediate probes
_NPROBE = 10

_prog_cache = {}


def _build_program():
    import concourse.bacc as bacc
    import concourse.tile as tile
    from concourse import mybir

    dt = mybir.dt
    f32 = dt.float32
    bf16 = dt.bfloat16
    AF = mybir.ActivationFunctionType

    nc = bacc.Bacc("TRN2", target_bir_lowering=False, debug=False)

    # grouped inputs (one dma_start each, issued in priority order)
    g0_d = nc.dram_tensor("g0", [128, DC * NKV + DC * D], bf16,
                          kind="ExternalInput").ap()
    g1_d = nc.dram_tensor("g1", [128, DC * N + DC * D], bf16, kind="ExternalInput").ap()
    g2_d = nc.dram_tensor("g2", [128, DC * D], bf16, kind="ExternalInput").ap()
    g3_d = nc.dram_tensor("g3", [128, DC * D], bf16, kind="ExternalInput").ap()
    g4_d = nc.dram_tensor("g4", [128, G4_W], f32, kind="ExternalInput").ap()
    y_d = nc.dram_tensor("y", [N, D], bf16, kind="ExternalOutput").ap()
    dbg_d = (nc.dram_tensor("dbg", [128, 512 * _NPROBE], f32,
                            kind="ExternalOutput").ap() if _DEBUG else None)

    with tile.TileContext(nc) as tc, \
         nc.allow_low_precision(reason="bf16 matmul streams, f32 accumulate"):
        with tc.tile_pool(name="const", bufs=1) as cpool:
            G0 = cpool.tile([128, DC * NKV + DC * D], bf16, name="G0")
            G1 = cpool.tile([128, DC * N + DC * D], bf16, name="G1")
            G2 = cpool.tile([128, DC * D], bf16, name="G2")
            G3 = cpool.tile([128, DC * D], bf16, name="G3")
            G4 = cpool.tile([128, G4_W], f32, name="G4")

            def xkT(c2):  # [128, NKV] slice for D-chunk c2
                return G0[:, NKV * c2:NKV * (c2 + 1)]

            def wk(c2):   # [128, D]
                return G0[:, DC * NKV + D * c2:DC * NKV + D * (c2 + 1)]

            def xT(c2):   # [128, N]
                return G1[:, N * c2:N * (c2 + 1)]

            def wq(c2):   # [128, D]
                return G1[:, DC * N + D * c2:DC * N + D * (c2 + 1)]

            def wv(c2):
                return G2[:, D * c2:D * (c2 + 1)]

            def wo(c2):
                return G3[:, D * c2:D * (c2 + 1)]

            # persistent result tiles
            kT_t = [cpool.tile([128, NKV], bf16, name=f"kT{c}")
                    for c in range(DC)]
            qT_t = [cpool.tile([128, N], bf16, name=f"qT{c}")
                    for c in range(DC)]
            v_t = [cpool.tile([128, D], bf16, name=f"v{c}")
                   for c in range(KC)]
            aoT_t = [cpool.tile([128, N], bf16, name=f"aoT{c}")
                     for c in range(DC)]
            zeros = cpool.tile([128, 128], bf16, name="zeros")
            ones_kv = cpool.tile([128, 1], bf16, name="ones_kv")
            ones97 = cpool.tile([97, 64], f32, name="ones97")
            dum = cpool.tile([1, 32], f32, name="dum")
            dbg = (cpool.tile([128, 512 * _NPROBE], f32, name="dbg")
                   if _DEBUG else None)

            def probe(k, src, rows=128, cols=512, row0=0):
                if _DEBUG:
                    r1 = row0 + 1 if rows is None else rows
                    nc.vector.tensor_scalar_add(
                        dbg[row0:r1, 512 * k:512 * k + cols], src, 0.0)

            # ---- issue everything up front ----
            nc.vector.memset(zeros[:], 0.0)
            nc.vector.memset(ones_kv[:], 1.0)
            nc.vector.memset(ones97[:], 1.0)
            if _DEBUG:
                nc.vector.memset(dbg[:], 0.0)
            nc.sync.dma_start(G0[:], g0_d[:, :])
            nc.sync.dma_start(G1[:], g1_d[:, :])
            nc.sync.dma_start(G2[:], g2_d[:, :])
            nc.sync.dma_start(G3[:], g3_d[:, :])
            nc.sync.dma_start(G4[:], g4_d[:, :])
            # preload the exp table set while DMA is in flight
            nc.scalar.activation(dum[:], zeros[0:1, 0:32], AF.Exp, scale=1.0)

            # ---- PE warmup: open the HAM clock gate during DMA flight ----
            with tc.tile_pool(name="wp", bufs=1, space="PSUM") as wp:
                wps = wp.tile([128, 512], f32, name="wps")
                for _ in range(32):
                    nc.tensor.matmul(wps[:, 0:128], zeros[:], zeros[:],
                                     start=True, stop=True)

            # ---- k projection (no bias: cancels in softmax) ----
            with tc.tile_pool(name="kpp", bufs=2, space="PSUM") as kpp:
                for dp in range(DC):
                    ps = kpp.tile([128, NKV], f32, name="kps")
                    for c2 in range(DC):
                        lhs = wk(c2)[:, 128 * dp:128 * (dp + 1)]
                        nc.tensor.matmul(
                            ps[:, 0:512], lhs, xkT(c2)[:, 0:512],
                            start=(c2 == 0), stop=(c2 == DC - 1))
                        nc.tensor.matmul(
                            ps[:, 512:NKV], lhs, xkT(c2)[:, 512:NKV],
                            start=(c2 == 0), stop=(c2 == DC - 1))
                    nc.scalar.copy(kT_t[dp][:], ps[:])

            # ---- q projection for head pair 0 (rest are fillers) ----
            with tc.tile_pool(name="qpp", bufs=2, space="PSUM") as qpp:
                for hf in range(2):
                    ps = qpp.tile([128, 512], f32, name="qps")
                    for c2 in range(DC):
                        nc.tensor.matmul(
                            ps[:], wq(c2)[:, 0:128],
                            xT(c2)[:, 512 * hf:512 * (hf + 1)],
                            start=(c2 == 0), stop=(c2 == DC - 1))
                    nc.vector.tensor_scalar_add(
                        qT_t[0][:, 512 * hf:512 * (hf + 1)], ps[:],
                        G4[:, G4_BQ:G4_BQ + 1])

            # ---- attention over head pairs, projections as fillers ----
            with tc.tile_pool(name="scp", bufs=2, space="PSUM") as scp, \
                 tc.tile_pool(name="oap", bufs=2, space="PSUM") as oap, \
                 tc.tile_pool(name="dnp", bufs=1, space="PSUM") as dnp, \
                 tc.tile_pool(name="flp", bufs=1, space="PSUM") as flp, \
                 tc.tile_pool(name="pp", bufs=4) as pp, \
                 tc.tile_pool(name="rcp", bufs=4) as rcp:

                def fill_vproj(c):
                    ps = flp.tile([128, 512], f32, name="fps")
                    for c2 in range(DC):
                        nc.tensor.matmul(
                            ps[:], xkT(c2)[:, 128 * c:128 * (c + 1)],
                            wv(c2)[:], start=(c2 == 0), stop=(c2 == DC - 1))
                    nc.vector.tensor_scalar_add(v_t[c][:], ps[:], 0.0)

                def fill_qproj(dp, hf):
                    ps = flp.tile([128, 512], f32, name="fps")
                    for c2 in range(DC):
                        nc.tensor.matmul(
                            ps[:], wq(c2)[:, 128 * dp:128 * (dp + 1)],
                            xT(c2)[:, 512 * hf:512 * (hf + 1)],
                            start=(c2 == 0), stop=(c2 == DC - 1))
                    nc.vector.tensor_scalar_add(
                        qT_t[dp][:, 512 * hf:512 * (hf + 1)], ps[:],
                        G4[:, G4_BQ + dp:G4_BQ + dp + 1])

                # filler schedule keyed by (dp, c): v just-in-time for av,
                # q one head-pair ahead of its score matmuls
                fillers = {
                    (0, 0): [lambda: fill_vproj(0), lambda: fill_qproj(1, 0)],
                    (0, 1): [lambda: fill_vproj(1), lambda: fill_qproj(1, 1)],
                    (0, 2): [lambda: fill_vproj(2)],
                    (0, 3): [lambda: fill_vproj(3)],
                    (0, 4): [lambda: fill_vproj(4)],
                    (1, 0): [lambda: fill_qproj(2, 0)],
                    (1, 1): [lambda: fill_qproj(2, 1)],
                    (1, 2): [lambda: fill_qproj(3, 0)],
                    (1, 3): [lambda: fill_qproj(3, 1)],
                }

                for dp in range(DC):
                    oa = [oap.tile([128, 512], f32, name="oa")
                          for hf in range(2)]
                    den = dnp.tile([128, 512], f32, name="den")
                    p_t = []  # per chunk: [p_hf0, p_hf1]

                    def scores(c, hf):
                        sc = scp.tile([128, N], f32, name="sc")
                        for hi in range(2):
                            nc.tensor.matmul(
                                sc[:, 512 * hi:512 * (hi + 1)],
                                kT_t[dp][HD * hi:HD * (hi + 1),
                                         128 * c:128 * (c + 1)],
                                qT_t[dp][HD * hi:HD * (hi + 1),
                                         512 * hf:512 * (hf + 1)],
                                start=True, stop=True)
                        p = pp.tile([128, N], bf16, name="p")
                        nc.scalar.activation(
                            p[:], sc[:], AF.Exp,
                            bias=G4[:, G4_EXPB + c:G4_EXPB + c + 1],
                            scale=SCALE)
                        if _DEBUG and dp == 0 and c == 0 and hf == 0:
                            probe(5, sc[:, 0:512])
                            probe(6, p[:, 0:512])
                        return p

                    def av(c, hf):
                        # col-packed pair: head A -> partitions 0:64,
                        # head B -> 64:128 of one PSUM bank. Each writer is
                        # its own accumulation group over c (per-partition
                        # zero regions), hence skip_group_check.
                        for hi in range(2):
                            nc.tensor.matmul(
                                oa[hf][64 * hi:64 * (hi + 1), :],
                                v_t[c][:, 64 * (2 * dp + hi):64 * (2 * dp + hi) + 64],
                                p_t[c][hf][:, 512 * hi:512 * (hi + 1)],
                                start=(c == 0), stop=(c == KC - 1),
                                tile_position=(0, 64 * hi),
                                skip_group_check=True)
                        # denominators: M=1 ones-matmuls, col groups
                        # 0/32 (hf0) and 64/96 (hf1) run concurrently
                        for hi in range(2):
                            r = 64 * hf + 32 * hi
                            nc.tensor.matmul(
                                den[r:r + 1, :], ones_kv[:],
                                p_t[c][hf][:, 512 * hi:512 * (hi + 1)],
                                start=(c == 0), stop=(c == KC - 1),
                                tile_position=(0, r),
                                skip_group_check=True)

                    for c in range(KC):
                        ps = []
                        for hf in range(2):
                            ps.append(scores(c, hf))
                            if c >= 1:
                                av(c - 1, hf)
                        p_t.append(ps)
                        for f in fillers.get((dp, c), ()):
                            f()
                    for hf in range(2):
                        av(KC - 1, hf)

                    # normalize: stage + recip the four denominator rows
                    # (SBUF start partitions must be 0/32/64/96), then
                    # partition-broadcast and one multiply per query half
                    # custom DVE ops misbehave on HW when the AP starts at a
                    # non-zero partition: memset the staging tile and run one
                    # reciprocal over all 97 partitions from partition 0
                    db = rcp.tile([97, 512], f32, name="db")
                    rc = rcp.tile([97, 512], f32, name="rc")
                    nc.vector.memset(db[:], 1.0)
                    for j in range(4):
                        r = 32 * j
                        nc.vector.tensor_scalar_add(
                            db[r:r + 1, :], den[r:r + 1, :], 0.0)
                    nc.vector.reciprocal_approx_fast(rc[:], db[:])
                    if _DEBUG and dp == 0:
                        for j in range(4):
                            r = 32 * j
                            probe(0, db[r:r + 1, :], rows=None, row0=r)
                            probe(1, rc[r:r + 1, :], rows=None, row0=r)
                    for hf in range(2):
                        # partition-broadcast the two reciprocal rows via
                        # col-packed K=1 matmuls (gpsimd partition_broadcast
                        # cannot write partitions 64:128 on HW)
                        rbs = flp.tile([128, 512], f32, name="fps")
                        for hi in range(2):
                            r = 64 * hf + 32 * hi
                            nc.tensor.matmul(
                                rbs[64 * hi:64 * (hi + 1), :],
                                ones97[r:r + 1, :], rc[r:r + 1, :],
                                start=True, stop=True,
                                tile_position=(r, 64 * hi),
                                skip_group_check=True)
                        rbs_sb = rcp.tile([128, 512], f32, name="rbs_sb")
                        nc.vector.tensor_scalar_add(rbs_sb[:], rbs[:], 0.0)
                        if _DEBUG and dp == 0 and hf == 0:
                            probe(2, rbs_sb[:])
                            probe(3, oa[0][:])
                        nc.vector.tensor_mul(
                            aoT_t[dp][:, 512 * hf:512 * (hf + 1)],
                            oa[hf][:], rbs_sb[:])
                    if _DEBUG and dp == 0:
                        probe(4, aoT_t[0][:, 0:512])
                        probe(7, kT_t[0][:, 0:512])
                        probe(8, qT_t[0][:, 0:512])
                        probe(9, v_t[0][:, 0:512])

            # ---- output projection, bf16 writeback ----
            with tc.tile_pool(name="ypp", bufs=2, space="PSUM") as ypp, \
                 tc.tile_pool(name="ysp", bufs=2) as ysp:
                for ic in range(N // 128):
                    yps = ypp.tile([128, D], f32, name="yps")
                    for dp in range(DC):
                        nc.tensor.matmul(
                            yps[:], aoT_t[dp][:, 128 * ic:128 * (ic + 1)],
                            wo(dp)[:], start=(dp == 0), stop=(dp == DC - 1))
                    ysb = ysp.tile([128, D], bf16, name="ysb")
                    nc.vector.tensor_add(ysb[:], yps[:],
                                         G4[:, G4_BOB:G4_BOB + D])
                    nc.sync.dma_start(y_d[128 * ic:128 * (ic + 1), :], ysb[:])
                if _DEBUG:
                    nc.sync.dma_start(dbg_d[:, :], dbg[:])

    return nc


def _get_program():
    if "nc" not in _prog_cache:
        nc = _build_program()
        if not nc.is_finalized():
            nc.finalize()
        _prog_cache["nc"] = nc
    return _prog_cache["nc"]


def _prep_core(b, x, mask, wq, bq, wk, bk, wv, bv, wo, bo):
    import ml_dtypes

    b16 = ml_dtypes.bfloat16
    f = np.float32
    xb = np.ascontiguousarray(x[b], dtype=f)                # [N, D]
    idx = np.nonzero(mask[b])[0]
    nv = int(idx.size)
    assert 1 <= nv <= NKV, f"batch {b}: {nv} valid keys, NKV={NKV}"
    xk = np.zeros((NKV, D), f)
    xk[:nv] = xb[idx]
    xkT = np.ascontiguousarray(xk.T)                        # [D, NKV]
    xT = np.ascontiguousarray(xb.T)                         # [D, N]

    def chunks(a):  # [D, W] -> [128, DC*W]
        return np.concatenate([a[128 * c:128 * (c + 1), :]
                               for c in range(DC)], axis=1)

    g0 = np.concatenate([chunks(xkT), chunks(wk.astype(f))], axis=1)
    g1 = np.concatenate([chunks(xT), chunks(wq.astype(f))], axis=1)
    g2 = chunks(wv.astype(f))
    g3 = chunks(wo.astype(f))

    pos = np.arange(128)[:, None] + 128 * np.arange(KC)[None, :]
    expb = np.where(pos < nv, 0.0, PAD_BIAS).astype(f)      # [128, KC]
    bqc = np.stack([bq.astype(f)[128 * c:128 * (c + 1)]
                    for c in range(DC)], axis=1)            # [128, DC]
    bob = (bo.astype(f) + bv.astype(f) @ wo.astype(f)).reshape(D)
    g4 = np.concatenate([bqc, expb,
                         np.broadcast_to(bob, (128, D))], axis=1)
    return {
        "g0": np.ascontiguousarray(g0).astype(b16),
        "g1": np.ascontiguousarray(g1).astype(b16),
        "g2": np.ascontiguousarray(g2).astype(b16),
        "g3": np.ascontiguousarray(g3).astype(b16),
        "g4": np.ascontiguousarray(g4, f),
    }


def _run(inputs):
    import os

    os.environ["BASS_NEVER_TRACE"] = "1"
    from concourse.bass_utils import run_bass_kernel_spmd

    nc = _get_program()
    in_maps = [_prep_core(b, **inputs) for b in range(B)]
    res = run_bass_kernel_spmd(nc, in_maps, core_ids=list(range(B)),
                               trace=False)
    out = np.stack([res.results[b]["y"] for b in range(B)], axis=0)
    return out.astype(np.float32), res


def kernel(**inputs) -> np.ndarray:
    out, _ = _run(inputs)
    return out
